# revision 6
# baseline (speedup 1.0000x reference)
"""Trainium2 Bass kernel for nn_FGNet (gnn_message_passing).

Strategy (v3)
-------------
Per-edge weights are gathers from tiny tables (169 edge types), so edges are
sorted by type id and processed in uniform 256-edge blocks (one id per block,
padded; 2 segments x 128 edges).  Device math per block, all bf16 in / f32
accumulate:

    t_h   = relu(W_id.T @ feats_h + b_id)        h = 0,1   (K=64 matmul)
    p_h,i = prod_{j != i} t_h,j                  3 DVE muls, bf16
    msg_i = ho_id,i.T @ [p_0,i | p_1,i]          K=128, N=256 matmul
    ps2 -> m (bf16) copy on gpsimd/vector/scalar round-robin, DMA out
    (the second bias b2 is linear in the segment-sum -> folded on the host)

bf16 matmuls run at 1 col/cycle at full PE p-state and halve all DMA bytes
vs the f32r baseline (end-to-end rel err ~2e-3, tolerance 2e-2).  The ho
tables for all B blocks are hoisted into SBUF once ([128, B*192] bf16), W
streams with the feats (one [64, 2, 896] DMA per block pair), outputs are
written bf16 one DMA per block pair.  Empirical HW constraints inherited
from the baseline: matmul *input* partition offsets crash the runtime, so
all matmul operands start at partition 0; PSUM cannot be DMA'd directly, so
MM2 results are copied (and converted) to SBUF first.

Packed layouts per core (B blocks):
    fz   [B//2, 64, 2, 896] bf16   feats col h*384 + i*128 + e (l=partition),
                                   then W_id (lhsT [64, 128]) in cols 768:896
    hot  [128, B*192] bf16         col b*192 + i*64 + l, partition r
    bia  [128, B] f32              bias_p[id].T per block
    msgs [B//2, 64, 2, 768] bf16   row l, col i*256 + h*128 + e

Host side (vectorized numpy): id computation, sort, feature gather, packing,
unpermute, b2 bias add and the final segment-sum into node_msg.
"""

import numpy as np

_BLK = 256          # edge slots per block (2 segments x 128)
_SEG = 128
_NCORES = 8

_prog_cache = {}


def _build_program(B):
    """Build the SPMD device program for B blocks per core (B even)."""
    import concourse.mybir as mybir
    import concourse.tile as tile
    from concourse import bacc

    F32 = mybir.dt.float32
    BF16 = mybir.dt.bfloat16
    Relu = mybir.ActivationFunctionType.Relu

    assert B % 2 == 0

    nc = bacc.Bacc()
    fz = nc.declare_dram_parameter("fz", [B // 2, 64, 2, 896], BF16,
                                   isOutput=False)
    hot = nc.declare_dram_parameter("hot", [128, B * 192], BF16,
                                    isOutput=False)
    bia = nc.declare_dram_parameter("bia", [128, B], F32, isOutput=False)
    msgs = nc.declare_dram_parameter("msgs", [B // 2, 64, 2, 768], BF16,
                                     isOutput=True)

    with tile.TileContext(nc) as tc:
        with (
            tc.tile_pool(name="const", bufs=1) as const,
            tc.tile_pool(name="work", bufs=4) as work,
            tc.tile_pool(name="outp", bufs=3) as outp,
            tc.tile_pool(name="psum", bufs=2, space="PSUM") as psum,
        ):
            bt = const.tile([128, B], F32, name="bt")
            nc.sync.dma_start(out=bt[:], in_=bia[:])
            ht = const.tile([128, B * 192], BF16, name="ht")
            # split the big table load so block 0 can start early
            nsplit = 4
            step = (B + nsplit - 1) // nsplit
            for s in range(0, B, step):
                e = min(s + step, B)
                nc.sync.dma_start(out=ht[:, s * 192:e * 192],
                                  in_=hot[:, s * 192:e * 192])

            # Software-pipelined emission.  At iteration `it` each engine
            # works on a different block, so no engine waits on work emitted
            # in the same iteration:
            #   PE: MM1(it), MM2(it-2)   S: act(it-1)[, copy(it-3)]
            #   V: [copy(it-3),] muls(it-1)   G: muls(it-1)
            # fz is prefetched two pairs ahead; out-DMA per pair fires once
            # both halves are copied (it-3 odd).
            fks, ps1s, ts, ps, ps2s, ms = {}, {}, {}, {}, {}, {}

            def dma_in(q):
                fk = work.tile([64, 2, 896], BF16, name="fk", tag="fk")
                nc.sync.dma_start(out=fk[:], in_=fz[q])
                fks[q] = fk

            dma_in(0)
            dma_in(1)
            for it in range(B + 4):
                if it % 2 == 0 and it // 2 + 2 < B // 2:
                    dma_in(it // 2 + 2)
                b1 = it
                if b1 < B:
                    fk = fks[b1 // 2]
                    g = b1 % 2
                    ps1 = psum.tile([128, 2, 512], F32, name="ps1", tag="ps1")
                    for h in range(2):
                        nc.tensor.matmul(out=ps1[:, h, 0:384],
                                         lhsT=fk[:, g, 768:896],
                                         rhs=fk[:, g, h * 384:(h + 1) * 384],
                                         start=True, stop=True)
                    ps1s[b1] = ps1
                bA = it - 1
                if 0 <= bA < B:
                    t = work.tile([128, 2, 384], BF16, name="t", tag="t")
                    nc.scalar.activation(out=t[:], in_=ps1s.pop(bA)[:, :, 0:384],
                                         func=Relu, bias=bt[:, bA:bA + 1],
                                         scale=1.0)
                    ts[bA] = t
                bC = it - 3
                if 0 <= bC < B:
                    # GPSIMD cannot read PSUM on TRN2: psum->sbuf copies go
                    # to DVE with every 5th on the scalar engine.
                    src = ps2s.pop(bC)[:].rearrange("l i he -> l (i he)")
                    q, g = divmod(bC, 2)
                    if bC % 5 == 2:
                        nc.scalar.copy(out=ms[q][:, g, :], in_=src)
                    else:
                        nc.vector.tensor_copy(out=ms[q][:, g, :], in_=src)
                if 0 <= bA < B:
                    t = ts.pop(bA)
                    p = work.tile([128, 3, 2, 128], BF16, name="p", tag="p")
                    # 2 of 3 blocks' product muls run on GPSIMD, rest on DVE
                    eng = nc.vector if bA % 3 == 0 else nc.gpsimd
                    for i, (j, k) in enumerate(((1, 2), (0, 2), (0, 1))):
                        eng.tensor_mul(
                            out=p[:, i],
                            in0=t[:, :, 128 * j:128 * (j + 1)],
                            in1=t[:, :, 128 * k:128 * (k + 1)])
                    ps[bA] = p
                b2 = it - 2
                if 0 <= b2 < B:
                    if b2 % 2 == 0:
                        ms[b2 // 2] = outp.tile([64, 2, 768], BF16, name="m",
                                                tag="m")
                    p = ps.pop(b2)
                    ps2 = psum.tile([64, 3, 256], F32, name="ps2", tag="ps2")
                    for i in range(3):
                        nc.tensor.matmul(
                            out=ps2[:, i, :],
                            lhsT=ht[:, b2 * 192 + i * 64:b2 * 192 + (i + 1) * 64],
                            rhs=p[:, i].rearrange("r h e -> r (h e)"),
                            start=True, stop=True)
                    ps2s[b2] = ps2
                if 0 <= bC < B and bC % 2 == 1:
                    q = bC // 2
                    nc.sync.dma_start(out=msgs[q], in_=ms.pop(q)[:])
    nc.finalize()
    return nc


def _get_program(B):
    if B not in _prog_cache:
        _prog_cache[B] = _build_program(B)
    return _prog_cache[B]


def _prepare(x, nodes, fact, params, bias_p, ho_params, ho_bias):
    """Host-side: sort by id, build per-block packed arrays."""
    import ml_dtypes
    bf16 = ml_dtypes.bfloat16

    N, L = nodes.shape
    E = fact.shape[0]
    R = params.shape[2]
    NP = params.shape[0]           # 169
    MA = int(round(NP ** 0.5))     # 13

    ids = (x[fact[:, 0], 1] * MA + x[fact[:, 0], 2]).astype(np.int64)   # [E]
    perm = np.argsort(ids, kind="stable")
    ids_s = ids[perm]
    fact_s = fact[perm].astype(np.int64)                                 # [E,3]

    counts = np.bincount(ids_s, minlength=NP)                            # [NP]
    nblk = (counts + _BLK - 1) // _BLK                                   # [NP]
    blk_ids = np.repeat(np.arange(NP), nblk)                             # [NB]
    NB = int(blk_ids.shape[0])
    B = (NB + _NCORES - 1) // _NCORES
    if B % 2:
        B += 1
    NB8 = B * _NCORES
    blk_ids = np.concatenate([blk_ids, np.zeros(NB8 - NB, np.int64)])

    # slot -> sorted-edge-position map (-1 = padding)
    padded = nblk * _BLK
    pad_off = np.concatenate([[0], np.cumsum(padded)])
    off = np.concatenate([[0], np.cumsum(counts)])
    total = int(pad_off[-1])
    t_of = np.repeat(np.arange(NP), padded)
    jloc = np.arange(total) - pad_off[t_of]
    src = np.where(jloc < counts[t_of], off[t_of] + jloc, -1)
    src = np.concatenate([src, np.full(NB8 * _BLK - total, -1, np.int64)])
    valid = src >= 0

    # gather features per slot
    nf = nodes[fact_s].astype(bf16)                                      # [E,3,L]
    featp = np.zeros((NB8 * _BLK, 3, L), bf16)
    featp[valid] = nf[src[valid]]

    # fz: feats (cols h*384 + i*128 + e over partitions l) + W (cols 768:896)
    fz = np.zeros((NB8, 64, 896), bf16)
    fz[:, :, 0:768] = (
        featp.reshape(NB8, 2, _SEG, 3, L).transpose(0, 4, 1, 3, 2)
        .reshape(NB8, 64, 768)
    )
    fz[:, :, 768:896] = params[blk_ids].astype(bf16)                     # [NB8,L,R]
    fz = fz.reshape(_NCORES, B // 2, 2, 64, 896).transpose(0, 1, 3, 2, 4)
    fz = np.ascontiguousarray(fz)                                        # [8,B/2,64,2,896]

    # hot: [128, B*192], col b*192 + i*64 + l
    hot = (
        ho_params[:, blk_ids].astype(bf16).transpose(1, 2, 0, 3)
        .reshape(NB8, R, 3 * L)
    )                                                                    # [NB8,128,192]
    hot = hot.reshape(_NCORES, B, R, 3 * L).transpose(0, 2, 1, 3)
    hot = np.ascontiguousarray(hot.reshape(_NCORES, R, B * 3 * L))       # [8,128,B*192]

    biasT = bias_p[blk_ids, 0].astype(np.float32)                        # [NB8,R]
    biasT = biasT.reshape(_NCORES, B, R).transpose(0, 2, 1)              # [8,R,B]

    return dict(fz=fz, hot=hot, biasT=np.ascontiguousarray(biasT), B=B,
                NB8=NB8, src=src, valid=valid, fact_s=fact_s, ids_s=ids_s,
                N=N, E=E, L=L)


def _postprocess(msgs_all, prep, ho_bias):
    """Decode per-slot messages, add host-side b2, segment-sum into node_msg."""
    NB8, N, E, L = prep["NB8"], prep["N"], prep["E"], prep["L"]
    src, valid, fact_s, ids_s = prep["src"], prep["valid"], prep["fact_s"], prep["ids_s"]
    # msgs_all [NB8, 64, 768] f32: row = l, col = i*256 + h*128 + e
    slots = (
        msgs_all.reshape(NB8, 64, 3, 2, _SEG).transpose(0, 3, 4, 2, 1)
        .reshape(NB8 * _BLK, 3, 64)
    )
    msg_e = np.empty((E, 3, L), np.float32)
    msg_e[src[valid]] = slots[valid]

    # fold in the second bias (linear in the segment-sum)
    msg_e += ho_bias[:, ids_s, 0].astype(np.float32).transpose(1, 0, 2)  # [E,3,L]

    idx_all = fact_s.T.reshape(-1)                                       # [3E]
    val_all = msg_e.transpose(1, 0, 2).reshape(-1, L)                    # [3E,L]
    order = np.argsort(idx_all, kind="stable")
    idx_sorted = idx_all[order]
    val_sorted = val_all[order]
    uniq, starts = np.unique(idx_sorted, return_index=True)
    sums = np.add.reduceat(val_sorted, starts, axis=0)
    out = np.zeros((N, L), np.float32)
    out[uniq] = sums
    return out


def _run_device(prep, trace=False, trace_kwargs=None):
    from concourse.bass_utils import run_bass_kernel_spmd

    B = prep["B"]
    nc = _get_program(B)
    in_maps = []
    for c in range(_NCORES):
        in_maps.append({
            "fz": prep["fz"][c],
            "hot": prep["hot"][c],
            "bia": prep["biasT"][c],
        })
    kwargs = {}
    if trace:
        kwargs["trace"] = True
        if trace_kwargs:
            kwargs.update(trace_kwargs)
    res = run_bass_kernel_spmd(nc, in_maps, list(range(_NCORES)), **kwargs)
    msgs_all = np.concatenate(
        [np.asarray(res.results[c]["msgs"]).astype(np.float32)
         .transpose(0, 2, 1, 3).reshape(-1, 64, 768)
         for c in range(_NCORES)], axis=0)
    return msgs_all, res


def kernel(x, nodes, fact, fact_dim, params, bias_p, ho_params, ho_bias,
           _trace=False, _trace_kwargs=None):
    x = np.asarray(x)
    nodes = np.asarray(nodes, dtype=np.float32)
    fact = np.asarray(fact)
    params = np.asarray(params)
    bias_p = np.asarray(bias_p)
    ho_params = np.asarray(ho_params)
    ho_bias = np.asarray(ho_bias)

    prep = _prepare(x, nodes, fact, params, bias_p, ho_params, ho_bias)
    msgs_all, res = _run_device(prep, trace=_trace, trace_kwargs=_trace_kwargs)
    out = _postprocess(msgs_all, prep, ho_bias)
    kernel.last_results = res
    return out
